# revision 6
# baseline (speedup 1.0000x reference)
"""Trainium2 Bass kernel for nn_DeltaFlowLoss (DeFlow-style scene-flow loss).

Strategy (data-parallel over points, 8 cores):
  - Each core streams its slice of points as [128 partitions, T point-columns].
  - Per point we compute: pts_loss, speed, finite-mask, speed-bucket flags,
    meta-class one-hots, and a masked instance id (invalid -> out-of-range).
  - A per-point-column 256-wide instance one-hot (DVE iota-compare, bf16) is
    contracted against 7 per-point "channel" rows on the TensorEngine,
    accumulating [7, 256] instance sums and [7, 6] bucket sums in PSUM.
  - The tiny per-core [7, 262] accumulator goes back to the host, which does
    the final scalar combination in numpy (exact reference semantics).

Self-contained: hardcodes shapes from the problem spec (N=4M points, K=256
instances, classes < 16, 8 cores).
"""

import sys
import numpy as np

sys.path.insert(0, "/opt/trn_rl_repo")

import ml_dtypes
from contextlib import ExitStack

import concourse.bass as bass
import concourse.bacc as bacc
import concourse.tile as tile
from concourse import mybir

F32 = mybir.dt.float32
BF16 = mybir.dt.bfloat16
I32 = mybir.dt.int32
Alu = mybir.AluOpType
Act = mybir.ActivationFunctionType

N_TOTAL = 4_000_000
N_CORES = 8
K_INST = 256
P = 128  # partitions

# Per-core grid: 128 partitions x T point-columns. 8*128*3904 = 3,997,696
# points on-device; the 2,304-point tail is folded in on the host.
T_FULL = 3904
TB_FULL = 488  # point-columns per block (8 blocks)

CLASS_WEIGHTS = np.array([0.1, 1.0, 2.0, 2.5, 1.5], dtype=np.float64)

# SY channel slot order (free dim of the SY tile).
# Stationary rows = slots 0..6, bucket-matmul moving cols = slots 5..10.
S_SPM, S_M0, S_M1, S_M2, S_M3, S_M, S_PLM, S_LO, S_PLLO, S_HI, S_PLHI = range(11)
NSY = 11
NS = 7   # stationary rows: [sp_m, moh0m, moh1m, moh2m, moh3m, m, pl_m]
NY = 6   # bucket cols:     [m, pl_m, lo, pl_lo, hi, pl_hi]
NOUT = K_INST + NY  # 262


def build_program(T=T_FULL, TB=TB_FULL, n_cores=N_CORES):
    """Emit the Bass program. Returns the compiled Bacc instance."""
    assert T % TB == 0
    nblocks = T // TB

    nc = bacc.Bacc("TRN2", target_bir_lowering=False, debug=False,
                   num_devices=n_cores)

    est_d = nc.dram_tensor("est", [P, T * 3], F32, kind="ExternalInput")
    gt_d = nc.dram_tensor("gt", [P, T * 3], F32, kind="ExternalInput")
    cls_d = nc.dram_tensor("cls", [P, T], I32, kind="ExternalInput")
    inst_d = nc.dram_tensor("inst", [P, T], I32, kind="ExternalInput")
    iota_d = nc.dram_tensor("iota", [P, K_INST], BF16, kind="ExternalInput")
    out_d = nc.dram_tensor("out", [NS, NOUT], F32, kind="ExternalOutput")

    with tile.TileContext(nc) as tc, ExitStack() as ctx:
        const_pool = ctx.enter_context(tc.tile_pool(name="const", bufs=1))
        in_pool = ctx.enter_context(tc.tile_pool(name="inp", bufs=2))
        work_pool = ctx.enter_context(tc.tile_pool(name="work", bufs=2))
        sy_pool = ctx.enter_context(tc.tile_pool(name="sy", bufs=2))
        oh_pool = ctx.enter_context(tc.tile_pool(name="oh", bufs=12))
        psum_pool = ctx.enter_context(
            tc.tile_pool(name="psum", bufs=1, space=bass.MemorySpace.PSUM))
        out_pool = ctx.enter_context(tc.tile_pool(name="outp", bufs=1))

        iota_t = const_pool.tile([P, K_INST], BF16)
        nc.sync.dma_start(iota_t[:], iota_d[:])

        # per-partition bias columns for ACT (float biases need const APs)
        biases = {}
        for bv in (512.0, -3.0, -8.5, -12.5):
            bt = const_pool.tile([P, 1], F32, tag=f"bias{bv}")
            nc.vector.memset(bt[:], bv)
            biases[bv] = bt

        ps_inst = psum_pool.tile([NS, K_INST], F32)  # instance sums
        ps_bkt = psum_pool.tile([NS, NY], F32)       # bucket-meta sums

        est_v = est_d.ap().rearrange("p (b t c) -> p b t c", b=nblocks, t=TB, c=3)
        gt_v = gt_d.ap().rearrange("p (b t c) -> p b t c", b=nblocks, t=TB, c=3)
        cls_v = cls_d.ap().rearrange("p (b t) -> p b t", b=nblocks, t=TB)
        inst_v = inst_d.ap().rearrange("p (b t) -> p b t", b=nblocks, t=TB)

        for b in range(nblocks):
            est = in_pool.tile([P, TB, 3], F32, tag="est")
            gt = in_pool.tile([P, TB, 3], F32, tag="gt")
            cls_i = in_pool.tile([P, TB], I32, tag="cls")
            inst_i = in_pool.tile([P, TB], I32, tag="inst")
            nc.sync.dma_start(est[:], est_v[:, b])
            nc.sync.dma_start(gt[:], gt_v[:, b])
            nc.sync.dma_start(cls_i[:], cls_v[:, b])
            nc.sync.dma_start(inst_i[:], inst_v[:, b])

            sy = sy_pool.tile([P, NSY, TB], BF16, tag="sy")

            # --- casts (ACT) ---
            cls_f = work_pool.tile([P, TB], F32, tag="clsf")
            nc.scalar.activation(cls_f[:], cls_i[:], Act.Copy, bias=0.0)
            instf512 = work_pool.tile([P, TB], F32, tag="instf")
            nc.scalar.activation(instf512[:], inst_i[:], Act.Identity, bias=biases[512.0][:])

            # --- norms ---
            diff = work_pool.tile([P, TB, 3], F32, tag="diff")
            nc.vector.tensor_tensor(diff[:], est[:], gt[:], Alu.subtract)
            d2 = work_pool.tile([P, TB, 3], F32, tag="d2")
            nc.scalar.activation(d2[:], diff[:], Act.Square)
            gt2 = work_pool.tile([P, TB, 3], F32, tag="gt2")
            nc.scalar.activation(gt2[:], gt[:], Act.Square)
            d2s = work_pool.tile([P, TB], F32, tag="d2s")
            nc.vector.tensor_reduce(d2s[:], d2[:], mybir.AxisListType.X, Alu.add)
            gt2s = work_pool.tile([P, TB], F32, tag="gt2s")
            nc.vector.tensor_reduce(gt2s[:], gt2[:], mybir.AxisListType.X, Alu.add)

            # pts_loss / speed (speed = sqrt(100*gt2s) = ||gt||/0.1),
            # written straight into their bf16 SY slots
            nc.scalar.activation(sy[:, S_PLM], d2s[:], Act.Sqrt)
            nc.scalar.activation(sy[:, S_SPM], gt2s[:], Act.Sqrt, scale=100.0)

            # --- finite mask (NaN/Inf -> 0) ---
            s2 = work_pool.tile([P, TB], F32, tag="s2")
            nc.vector.tensor_tensor(s2[:], d2s[:], gt2s[:], Alu.add)
            nc.vector.tensor_scalar(sy[:, S_M], s2[:], 3.0e38, None, Alu.is_lt)

            # masked instance id: invalid points -> 512+inst (no one-hot col)
            adj = work_pool.tile([P, TB], F32, tag="adj")
            nc.vector.scalar_tensor_tensor(
                adj[:], sy[:, S_M], -512.0, instf512[:], Alu.mult, Alu.add)

            # --- speed buckets (on squared norm; thresholds 0.04^2, 0.1^2) ---
            nc.vector.tensor_scalar(sy[:, S_LO], gt2s[:], 1.6e-3, None, Alu.is_lt)
            nc.vector.tensor_scalar(sy[:, S_HI], gt2s[:], 1.0e-2, None, Alu.is_gt)

            # --- meta one-hots (classes 0..15) ---
            # meta1 vehicle {7,8,9,10,12,13} = (|c-8.5|<=1.5) + (|c-12.5|==0.5)
            # meta2 ped {2,3,4} = |c-3|<=1 ; meta3 wheeled {6,11} = |c-8.5|==2.5
            a3 = work_pool.tile([P, TB], F32, tag="a3")
            nc.scalar.activation(a3[:], cls_f[:], Act.Abs, bias=biases[-3.0][:])
            a85 = work_pool.tile([P, TB], F32, tag="a85")
            nc.scalar.activation(a85[:], cls_f[:], Act.Abs, bias=biases[-8.5][:])
            a125 = work_pool.tile([P, TB], F32, tag="a125")
            nc.scalar.activation(a125[:], cls_f[:], Act.Abs, bias=biases[-12.5][:])

            nc.vector.tensor_scalar(sy[:, S_M0], cls_f[:], 0.0, None, Alu.is_equal)
            nc.vector.tensor_scalar(sy[:, S_M2], a3[:], 1.0, None, Alu.is_le)
            nc.vector.tensor_scalar(sy[:, S_M3], a85[:], 2.5, None, Alu.is_equal)
            va = work_pool.tile([P, TB], F32, tag="va")
            nc.vector.tensor_scalar(va[:], a85[:], 1.5, None, Alu.is_le)
            nc.vector.scalar_tensor_tensor(
                sy[:, S_M1], a125[:], 0.5, va[:], Alu.is_equal, Alu.add)

            nc.vector.tensor_tensor(sy[:, S_PLLO], sy[:, S_PLM], sy[:, S_LO],
                                    Alu.mult)
            nc.vector.tensor_tensor(sy[:, S_PLHI], sy[:, S_PLM], sy[:, S_HI],
                                    Alu.mult)

            # --- per-point-column one-hot + matmuls ---
            for t in range(TB):
                col = b * TB + t
                first = col == 0
                last = col == T - 1
                oh = oh_pool.tile([P, K_INST], BF16, tag="oh")
                nc.vector.tensor_scalar(
                    oh[:], iota_t[:], adj[:, t:t + 1], None, Alu.is_equal)
                nc.tensor.matmul(ps_inst[:], sy[:, 0:NS, t], oh[:],
                                 start=first, stop=last)
                nc.tensor.matmul(ps_bkt[:], sy[:, 0:NS, t], sy[:, S_M:S_M + NY, t],
                                 start=first, stop=last)

        out_sb = out_pool.tile([NS, NOUT], F32)
        nc.vector.tensor_copy(out_sb[:, 0:K_INST], ps_inst[:])
        nc.vector.tensor_copy(out_sb[:, K_INST:NOUT], ps_bkt[:])
        nc.sync.dma_start(out_d[:], out_sb[:])

    nc.compile()
    return nc


# ---------------------------------------------------------------------------
# Host-side helpers
# ---------------------------------------------------------------------------

def np_partials(est, gt, cls, inst, dtype=np.float64):
    """Numpy model of the device accumulator [NS, NOUT] for a set of points."""
    est = est.astype(dtype)
    gt = gt.astype(dtype)
    mask = np.isfinite(est).all(-1) & np.isfinite(gt).all(-1)
    pl = np.where(mask, np.sqrt(((est - gt) ** 2).sum(-1)), 0.0)
    sp = np.where(mask, np.sqrt((gt ** 2).sum(-1)) * 10.0, 0.0)
    g2 = np.where(mask, (gt ** 2).sum(-1), 0.0)
    m = mask.astype(dtype)
    lo = (g2 < 1.6e-3).astype(dtype)
    hi = (g2 > 1.0e-2).astype(dtype)

    e0 = (cls == 0)
    veh = np.isin(cls, [7, 8, 9, 10, 12, 13])
    ped = np.isin(cls, [2, 3, 4])
    whl = np.isin(cls, [6, 11])

    rows = np.stack([sp * m, e0 * m, veh * m, ped * m, whl * m, m, pl * m])
    acc = np.zeros((NS, NOUT), dtype)
    inst_m = np.where(mask, inst, K_INST)  # invalid -> no column
    ioh = np.zeros((len(m), K_INST + 1), dtype)
    ioh[np.arange(len(m)), inst_m] = 1.0
    acc[:, 0:K_INST] = rows @ ioh[:, 0:K_INST]
    ycols = np.stack([m, pl * m, lo, pl * m * lo, hi, pl * m * hi], axis=1)
    acc[:, K_INST:] = rows @ ycols
    return acc


def combine(acc):
    """acc [NS, NOUT] (summed over cores + tail) -> scalar loss (float64)."""
    sp_sum = acc[0, 0:K_INST]
    cnt = acc[5, 0:K_INST]
    pl_sum = acc[6, 0:K_INST]
    meta_cnt = np.zeros((K_INST, 5))
    for j in range(4):
        meta_cnt[:, j] = acc[1 + j, 0:K_INST]
    meta_cnt[:, 4] = cnt - meta_cnt[:, 0:4].sum(1)

    # bucket block: cols [cnt_tot, pl_tot, cnt_lo, pl_lo, cnt_hi, pl_hi]
    bkt = acc[:, K_INST:]

    def masked_mean(s, c):
        return s / c if c > 0 else 0.0

    def bucket_means(row):
        c_tot, p_tot, c_lo, p_lo, c_hi, p_hi = row
        c_mid = c_tot - c_lo - c_hi
        p_mid = p_tot - p_lo - p_hi
        return (masked_mean(p_lo, c_lo), masked_mean(p_mid, c_mid),
                masked_mean(p_hi, c_hi))

    mlo, mmid, mhi = bucket_means(bkt[5])
    base = mlo + mmid + mhi

    class_loss = 0.0
    all_row = bkt[5].copy()
    meta_rows = [bkt[1 + j] for j in range(4)]
    meta_rows.append(all_row - sum(meta_rows))
    for j in range(5):
        l, mm, h = bucket_means(meta_rows[j])
        class_loss += CLASS_WEIGHTS[j] * (0.1 * l + 0.4 * mm + 0.5 * h)

    safe_cnt = np.maximum(cnt, 1.0)
    sp_mean = sp_sum / safe_cnt
    ins_err = np.nan_to_num(pl_sum / safe_cnt, nan=0.0, posinf=0.0, neginf=0.0)
    mode_cls = np.argmax(meta_cnt, axis=1)
    valid = (np.arange(K_INST) > 0) & (cnt > 0) & (sp_mean > 0.4)
    contrib = ins_err * np.exp(ins_err) * CLASS_WEIGHTS[mode_cls]
    n_valid = valid.sum()
    inst_loss = (contrib * valid).sum() / max(n_valid, 1) if n_valid > 0 else 0.0

    return base + class_loss + inst_loss


_NC_CACHE = {}


def _get_program():
    key = (T_FULL, TB_FULL)
    if key not in _NC_CACHE:
        _NC_CACHE[key] = build_program()
    return _NC_CACHE[key]


def make_in_maps(est_flow, gt_flow, gt_classes, gt_instance,
                 T=T_FULL, n_cores=N_CORES):
    npc = P * T  # points per core
    iota_np = np.broadcast_to(
        np.arange(K_INST, dtype=ml_dtypes.bfloat16), (P, K_INST)).copy()
    in_maps = []
    for c in range(n_cores):
        s = slice(c * npc, (c + 1) * npc)
        in_maps.append({
            "est": np.ascontiguousarray(
                est_flow[s].reshape(P, T * 3).astype(np.float32)),
            "gt": np.ascontiguousarray(
                gt_flow[s].reshape(P, T * 3).astype(np.float32)),
            "cls": np.ascontiguousarray(
                gt_classes[s].reshape(P, T).astype(np.int32)),
            "inst": np.ascontiguousarray(
                gt_instance[s].reshape(P, T).astype(np.int32)),
            "iota": iota_np,
        })
    return in_maps


def kernel(est_flow, gt_flow, gt_classes, gt_instance, _results_hook=None):
    est_flow = np.asarray(est_flow)
    gt_flow = np.asarray(gt_flow)
    gt_classes = np.asarray(gt_classes)
    gt_instance = np.asarray(gt_instance)

    from concourse.bass_utils import run_bass_kernel_spmd

    nc = _get_program()
    in_maps = make_in_maps(est_flow, gt_flow, gt_classes, gt_instance)
    res = run_bass_kernel_spmd(nc, in_maps, core_ids=list(range(N_CORES)))
    if _results_hook is not None:
        _results_hook(res)

    acc = np.zeros((NS, NOUT), np.float64)
    for r in res.results:
        acc += r["out"].astype(np.float64)

    ndev = N_CORES * P * T_FULL
    if ndev < len(gt_classes):  # tail points on host
        s = slice(ndev, None)
        acc += np_partials(est_flow[s], gt_flow[s], gt_classes[s],
                           gt_instance[s])

    return np.float32(combine(acc))


# revision 13
# speedup vs baseline: 1.7769x; 1.7769x over previous
"""Trainium2 Bass kernel for nn_DeltaFlowLoss (DeFlow-style scene-flow loss).

Strategy (data-parallel over points, 8 cores):
  - Each core streams its slice of points as [128 partitions, T point-columns].
  - Per point: pts_loss, speed, finite-mask, speed-bucket flags, meta one-hots,
    and a masked instance id. Instance ids are split k = 128*h + r; channels
    are duplicated into h0/h1 row blocks so a 128-wide one-hot suffices.
  - Per point-column, a 128-wide instance one-hot (DVE iota-compare bf16;
    some columns built on the Scalar engine as relu(1-|iota-adj|)) is
    contracted with the 14 channel rows on the TensorEngine, accumulating
    [14, 128] instance sums and [14, 6] bucket sums in PSUM.
  - Per-core [14, 134] accumulators go to the host, which does the final
    scalar combination in numpy (exact reference semantics).

Self-contained: hardcodes shapes from the problem spec (N=4M points, K=256
instances, classes < 16, 8 cores).
"""

import sys
import numpy as np

sys.path.insert(0, "/opt/trn_rl_repo")

import ml_dtypes
from contextlib import ExitStack

import concourse.bass as bass
import concourse.bacc as bacc
import concourse.tile as tile
from concourse import mybir

F32 = mybir.dt.float32
BF16 = mybir.dt.bfloat16
I32 = mybir.dt.int32
Alu = mybir.AluOpType
Act = mybir.ActivationFunctionType

N_TOTAL = 4_000_000
N_CORES = 8
K_INST = 256
KH = 128  # one-hot width (instance ids mod 128)
P = 128   # partitions

# Per-core grid: 128 partitions x T point-columns. 8*128*3904 = 3,997,696
# points on-device; the 2,304-point tail is folded in on the host.
T_FULL = 3904
TB_FULL = 488   # point-columns per block (8 blocks)
GR = 8          # one-hot granule (columns per oh tile)
ACT_EVERY = 4   # (unused) legacy
# per-granule one-hot builder: G=gpsimd local_scatter, D=DVE iota-compare,
# A=ScalarE abs+relu
GRANULE_PATTERN = ["G", "D", "G", "D", "G", "A", "G", "G"]

CLASS_WEIGHTS = np.array([0.1, 1.0, 2.0, 2.5, 1.5], dtype=np.float64)

# Base channel slot order (free dim of the BASE tile). The first NCH slots are
# the per-half stationary channels; slots B_M..B_PLHI (consecutive) are the
# bucket-matmul moving columns.
B_SP, B_M0, B_M1, B_M2, B_M3, B_M, B_PL, B_LO, B_PLLO, B_HI, B_PLHI = range(11)
NB = 11
NCH = 7    # channels per half: [sp, moh0..moh3, m, pl]
NS = 14    # stationary rows: channels x {h0, h1}
NY = 6     # bucket cols: [m, pl, lo, pl*lo, hi, pl*hi] = slots B_M..B_PLHI
YS = B_M
# PSUM/host row meaning within a half:
R_SP, R_M0, R_M1, R_M2, R_M3, R_M, R_PL = range(NCH)


def build_program(T=T_FULL, TB=TB_FULL, n_cores=N_CORES):
    assert T % TB == 0 and TB % GR == 0
    nblocks = T // TB
    ngr = TB // GR

    nc = bacc.Bacc("TRN2", target_bir_lowering=False, debug=False,
                   num_devices=n_cores)

    est_d = nc.dram_tensor("est", [P, T * 3], F32, kind="ExternalInput")
    gt_d = nc.dram_tensor("gt", [P, T * 3], F32, kind="ExternalInput")
    cls_d = nc.dram_tensor("cls", [P, T], I32, kind="ExternalInput")
    inst_d = nc.dram_tensor("inst", [P, T], I32, kind="ExternalInput")
    iota_d = nc.dram_tensor("iota", [P, KH], BF16, kind="ExternalInput")
    toff_d = nc.dram_tensor("toff", [P, GR], F32, kind="ExternalInput")
    out_d = nc.dram_tensor("out", [NS, KH], F32, kind="ExternalOutput")
    outb_d = nc.dram_tensor("outb", [NS, NY], F32, kind="ExternalOutput")

    with tile.TileContext(nc) as tc, ExitStack() as ctx:
        const_pool = ctx.enter_context(tc.tile_pool(name="const", bufs=1))
        in_pool = ctx.enter_context(tc.tile_pool(name="inp", bufs=2))
        work_pool = ctx.enter_context(tc.tile_pool(name="work", bufs=2))
        sy_pool = ctx.enter_context(tc.tile_pool(name="sy", bufs=2))
        oh_pool = ctx.enter_context(tc.tile_pool(name="oh", bufs=6))
        psum_pool = ctx.enter_context(
            tc.tile_pool(name="psum", bufs=1, space=bass.MemorySpace.PSUM))
        out_pool = ctx.enter_context(tc.tile_pool(name="outp", bufs=1))

        iota_t = const_pool.tile([P, KH], BF16)
        nc.sync.dma_start(iota_t[:], iota_d[:])
        toff_t = const_pool.tile([P, GR], F32)
        nc.sync.dma_start(toff_t[:], toff_d[:])
        ones_t = const_pool.tile([P, GR], BF16)
        nc.vector.memset(ones_t[:], 1.0)

        biases = {}
        for bv in (640.0, -3.0, -8.5, -12.5, 1.0):
            bt = const_pool.tile([P, 1], F32, tag=f"bias{bv}")
            nc.vector.memset(bt[:], bv)
            biases[bv] = bt

        ps_inst = psum_pool.tile([NS, KH], F32)
        ps_bkt = psum_pool.tile([NS, NY], F32)

        est_v = est_d.ap().rearrange("p (b t c) -> p b t c", b=nblocks, t=TB, c=3)
        gt_v = gt_d.ap().rearrange("p (b t c) -> p b t c", b=nblocks, t=TB, c=3)
        cls_v = cls_d.ap().rearrange("p (b t) -> p b t", b=nblocks, t=TB)
        inst_v = inst_d.ap().rearrange("p (b t) -> p b t", b=nblocks, t=TB)

        for b in range(nblocks):
            est = in_pool.tile([P, TB, 3], F32, tag="est")
            gt = in_pool.tile([P, TB, 3], F32, tag="gt")
            cls_i = in_pool.tile([P, TB], I32, tag="cls")
            inst_i = in_pool.tile([P, TB], I32, tag="inst")
            nc.sync.dma_start(est[:], est_v[:, b])
            nc.sync.dma_start(gt[:], gt_v[:, b])
            nc.sync.dma_start(cls_i[:], cls_v[:, b])
            nc.sync.dma_start(inst_i[:], inst_v[:, b])

            base = work_pool.tile([P, NB, TB], BF16, tag="base")
            sy = sy_pool.tile([P, NS, TB], BF16, tag="sy")

            # --- casts (ACT) ---
            cls_f = work_pool.tile([P, TB], F32, tag="clsf")
            nc.scalar.activation(cls_f[:], cls_i[:], Act.Copy, bias=0.0)
            instf = work_pool.tile([P, TB], F32, tag="instf")  # inst + 640
            nc.scalar.activation(instf[:], inst_i[:], Act.Identity,
                                 bias=biases[640.0][:])

            # --- norms ---
            diff = work_pool.tile([P, TB, 3], F32, tag="diff")
            nc.vector.tensor_tensor(diff[:], est[:], gt[:], Alu.subtract)
            nc.scalar.activation(diff[:], diff[:], Act.Square)
            gt2 = work_pool.tile([P, TB, 3], F32, tag="gt2")
            nc.scalar.activation(gt2[:], gt[:], Act.Square)
            d2s = work_pool.tile([P, TB], F32, tag="d2s")
            nc.vector.tensor_reduce(d2s[:], diff[:], mybir.AxisListType.X, Alu.add)
            gt2s = work_pool.tile([P, TB], F32, tag="gt2s")
            nc.vector.tensor_reduce(gt2s[:], gt2[:], mybir.AxisListType.X, Alu.add)

            # pts_loss / speed (= ||gt||/0.1 = sqrt(100*gt2s))
            nc.scalar.activation(base[:, B_PL], d2s[:], Act.Sqrt)
            nc.scalar.activation(base[:, B_SP], gt2s[:], Act.Sqrt, scale=100.0)

            # --- finite mask ---
            nc.vector.tensor_tensor(d2s[:], d2s[:], gt2s[:], Alu.add)
            nc.vector.tensor_scalar(base[:, B_M], d2s[:], 3.0e38, None, Alu.is_lt)

            # h1 = (inst >= 128); adjm = inst mod 128 for valid points,
            # in [-2048,-1921] for masked ones (negative => ignored by the
            # GPSIMD scatter; never equal to iota 0..127 elsewhere)
            h1 = work_pool.tile([P, TB], BF16, tag="h1")
            nc.vector.tensor_scalar(h1[:], instf[:], 768.0, None, Alu.is_ge)
            adjm = work_pool.tile([P, TB], F32, tag="adjm")
            nc.vector.scalar_tensor_tensor(
                adjm[:], h1[:], -128.0, instf[:], Alu.mult, Alu.add)
            nc.vector.tensor_scalar(adjm[:], adjm[:], -2688.0, None, Alu.add)
            nc.vector.scalar_tensor_tensor(
                adjm[:], base[:, B_M], 2048.0, adjm[:], Alu.mult, Alu.add)

            # --- speed buckets (on squared norm; 0.04^2 and 0.1^2) ---
            nc.vector.tensor_scalar(base[:, B_LO], gt2s[:], 1.6e-3, None, Alu.is_lt)
            nc.vector.tensor_scalar(base[:, B_HI], gt2s[:], 1.0e-2, None, Alu.is_gt)

            # --- meta one-hots (classes 0..15) ---
            # vehicle {7..10,12,13} = (|c-8.5|<=1.5)+(|c-12.5|==0.5)
            # ped {2,3,4} = |c-3|<=1 ; wheeled {6,11} = |c-8.5|==2.5
            a3 = work_pool.tile([P, TB], F32, tag="a3")
            nc.scalar.activation(a3[:], cls_f[:], Act.Abs, bias=biases[-3.0][:])
            a85 = work_pool.tile([P, TB], F32, tag="a85")
            nc.scalar.activation(a85[:], cls_f[:], Act.Abs, bias=biases[-8.5][:])
            a125 = work_pool.tile([P, TB], F32, tag="a125")
            nc.scalar.activation(a125[:], cls_f[:], Act.Abs, bias=biases[-12.5][:])

            nc.vector.tensor_scalar(base[:, B_M0], cls_f[:], 0.0, None, Alu.is_equal)
            nc.vector.tensor_scalar(base[:, B_M2], a3[:], 1.0, None, Alu.is_le)
            nc.vector.tensor_scalar(base[:, B_M3], a85[:], 2.5, None, Alu.is_equal)
            va = work_pool.tile([P, TB], F32, tag="va")
            nc.vector.tensor_scalar(va[:], a85[:], 1.5, None, Alu.is_le)
            nc.vector.scalar_tensor_tensor(
                base[:, B_M1], a125[:], 0.5, va[:], Alu.is_equal, Alu.add)

            nc.vector.tensor_tensor(base[:, B_PLLO], base[:, B_PL],
                                    base[:, B_LO], Alu.mult)
            nc.vector.tensor_tensor(base[:, B_PLHI], base[:, B_PL],
                                    base[:, B_HI], Alu.mult)

            # --- split channels into h0/h1 row blocks ---
            for i in range(NCH):
                nc.vector.tensor_tensor(sy[:, NCH + i], base[:, i], h1[:],
                                        Alu.mult)
                nc.vector.tensor_tensor(sy[:, i], base[:, i], sy[:, NCH + i],
                                        Alu.subtract)

            # --- per-column one-hot + matmuls ---
            for g in range(ngr):
                oh = oh_pool.tile([P, GR, KH], BF16, tag="oh")
                kind = GRANULE_PATTERN[g % len(GRANULE_PATTERN)]
                if kind == "G":
                    idx = oh_pool.tile([P, GR], mybir.dt.int16, tag="gidx")
                    nc.vector.tensor_tensor(
                        idx[:], adjm[:, g * GR:(g + 1) * GR], toff_t[:],
                        Alu.add)
                    nc.gpsimd.local_scatter(
                        oh[:], ones_t[:], idx[:], channels=P,
                        num_elems=GR * KH, num_idxs=GR)
                elif kind == "A":
                    # ScalarE path: |adjm - iota| then relu(1-x) granule-wide
                    for t in range(GR):
                        col = g * GR + t
                        nc.scalar.activation(
                            oh[:, t], iota_t[:], Act.Abs,
                            bias=adjm[:, col:col + 1], scale=-1.0)
                    nc.scalar.activation(
                        oh[:], oh[:], Act.Relu, bias=biases[1.0][:], scale=-1.0)
                else:
                    for t in range(GR):
                        col = g * GR + t
                        nc.vector.tensor_scalar(
                            oh[:, t], iota_t[:], adjm[:, col:col + 1],
                            None, Alu.is_equal)
                for t in range(GR):
                    col = g * GR + t
                    gcol = b * TB + col
                    nc.tensor.matmul(ps_inst[:], sy[:, 0:NS, col], oh[:, t],
                                     start=(gcol == 0), stop=(gcol == T - 1))
                    nc.tensor.matmul(
                        ps_bkt[:], sy[:, 0:NS, col],
                        base[:, YS:YS + NY, col],
                        start=(gcol == 0), stop=(gcol == T - 1))

        out_sb = out_pool.tile([NS, KH], F32)
        nc.vector.tensor_copy(out_sb[:], ps_inst[:])
        nc.sync.dma_start(out_d[:], out_sb[:])
        outb_sb = out_pool.tile([NS, NY], F32)
        nc.vector.tensor_copy(outb_sb[:], ps_bkt[:])
        nc.sync.dma_start(outb_d[:], outb_sb[:])

    nc.compile()
    return nc


# ---------------------------------------------------------------------------
# Host-side helpers
# ---------------------------------------------------------------------------

def np_partials(est, gt, cls, inst, dtype=np.float64):
    """Numpy model of the accumulators for a set of points (row order R_*)."""
    est = est.astype(dtype)
    gt = gt.astype(dtype)
    mask = np.isfinite(est).all(-1) & np.isfinite(gt).all(-1)
    pl = np.where(mask, np.sqrt(((est - gt) ** 2).sum(-1)), 0.0)
    sp = np.where(mask, np.sqrt((gt ** 2).sum(-1)) * 10.0, 0.0)
    g2 = np.where(mask, (gt ** 2).sum(-1), 0.0)
    m = mask.astype(dtype)
    lo = (g2 < 1.6e-3).astype(dtype)
    hi = (g2 > 1.0e-2).astype(dtype)

    e0 = (cls == 0)
    veh = np.isin(cls, [7, 8, 9, 10, 12, 13])
    ped = np.isin(cls, [2, 3, 4])
    whl = np.isin(cls, [6, 11])

    rows = np.stack([sp, e0 * 1.0, veh * 1.0, ped * 1.0, whl * 1.0, m, pl])
    inst_m = np.where(mask, inst, K_INST)
    ioh = np.zeros((len(m), K_INST + 1), dtype)
    ioh[np.arange(len(m)), inst_m] = 1.0
    acc_inst = rows @ ioh[:, 0:K_INST]
    ycols = np.stack([m, pl, lo, pl * lo, hi, pl * hi], axis=1)
    acc_bkt = rows @ ycols
    return {"inst": acc_inst, "bkt": acc_bkt}


def fold_device_out(out, outb):
    """Device out [NS,KH] + outb [4*NS,4*NY] -> {'inst','bkt'} (float64)."""
    out = out.astype(np.float64)
    inst = np.zeros((NCH, K_INST))
    inst[:, 0:KH] = out[0:NCH, 0:KH]
    inst[:, KH:K_INST] = out[NCH:NS, 0:KH]
    bkt14 = outb.astype(np.float64)
    bkt = bkt14[0:NCH] + bkt14[NCH:NS]
    return {"inst": inst, "bkt": bkt}


def combine(acc_inst, acc_bkt):
    """acc_inst [NCH, 256], acc_bkt [NCH, 6] -> scalar loss (float64)."""
    sp_sum = acc_inst[R_SP]
    cnt = acc_inst[R_M]
    pl_sum = acc_inst[R_PL]
    meta_cnt = np.zeros((K_INST, 5))
    for j in range(4):
        meta_cnt[:, j] = acc_inst[R_M0 + j]
    meta_cnt[:, 4] = cnt - meta_cnt[:, 0:4].sum(1)

    def masked_mean(s, c):
        return s / c if c > 0 else 0.0

    def bucket_means(row):
        c_tot, p_tot, c_lo, p_lo, c_hi, p_hi = row
        return (masked_mean(p_lo, c_lo),
                masked_mean(p_tot - p_lo - p_hi, c_tot - c_lo - c_hi),
                masked_mean(p_hi, c_hi))

    mlo, mmid, mhi = bucket_means(acc_bkt[R_M])
    base_loss = mlo + mmid + mhi

    class_loss = 0.0
    meta_rows = [acc_bkt[R_M0 + j] for j in range(4)]
    meta_rows.append(acc_bkt[R_M] - sum(meta_rows))
    for j in range(5):
        l, mm, h = bucket_means(meta_rows[j])
        class_loss += CLASS_WEIGHTS[j] * (0.1 * l + 0.4 * mm + 0.5 * h)

    safe_cnt = np.maximum(cnt, 1.0)
    sp_mean = sp_sum / safe_cnt
    ins_err = np.nan_to_num(pl_sum / safe_cnt, nan=0.0, posinf=0.0, neginf=0.0)
    mode_cls = np.argmax(meta_cnt, axis=1)
    valid = (np.arange(K_INST) > 0) & (cnt > 0) & (sp_mean > 0.4)
    contrib = ins_err * np.exp(ins_err) * CLASS_WEIGHTS[mode_cls]
    n_valid = valid.sum()
    inst_loss = (contrib * valid).sum() / max(n_valid, 1) if n_valid > 0 else 0.0

    return base_loss + class_loss + inst_loss


_NC_CACHE = {}


def _get_program():
    key = (T_FULL, TB_FULL)
    if key not in _NC_CACHE:
        _NC_CACHE[key] = build_program()
    return _NC_CACHE[key]


def make_in_maps(est_flow, gt_flow, gt_classes, gt_instance,
                 T=T_FULL, n_cores=N_CORES):
    npc = P * T
    iota_np = np.broadcast_to(
        np.arange(KH, dtype=ml_dtypes.bfloat16), (P, KH)).copy()
    in_maps = []
    for c in range(n_cores):
        s = slice(c * npc, (c + 1) * npc)
        in_maps.append({
            "est": np.ascontiguousarray(
                est_flow[s].reshape(P, T * 3).astype(np.float32)),
            "gt": np.ascontiguousarray(
                gt_flow[s].reshape(P, T * 3).astype(np.float32)),
            "cls": np.ascontiguousarray(
                gt_classes[s].reshape(P, T).astype(np.int32)),
            "inst": np.ascontiguousarray(
                gt_instance[s].reshape(P, T).astype(np.int32)),
            "iota": iota_np,
            "toff": np.broadcast_to(
                (np.arange(GR) * KH).astype(np.float32), (P, GR)).copy(),
        })
    return in_maps


def kernel(est_flow, gt_flow, gt_classes, gt_instance, _results_hook=None):
    est_flow = np.asarray(est_flow)
    gt_flow = np.asarray(gt_flow)
    gt_classes = np.asarray(gt_classes)
    gt_instance = np.asarray(gt_instance)

    from concourse.bass_utils import run_bass_kernel_spmd

    nc = _get_program()
    in_maps = make_in_maps(est_flow, gt_flow, gt_classes, gt_instance)
    res = run_bass_kernel_spmd(nc, in_maps, core_ids=list(range(N_CORES)))
    if _results_hook is not None:
        _results_hook(res)

    acc_inst = np.zeros((NCH, K_INST))
    acc_bkt = np.zeros((NCH, NY))
    for r in res.results:
        f = fold_device_out(r["out"], r["outb"])
        acc_inst += f["inst"]
        acc_bkt += f["bkt"]

    ndev = N_CORES * P * T_FULL
    if ndev < len(gt_classes):
        s = slice(ndev, None)
        t = np_partials(est_flow[s], gt_flow[s], gt_classes[s], gt_instance[s])
        acc_inst += t["inst"]
        acc_bkt += t["bkt"]

    return np.float32(combine(acc_inst, acc_bkt))


# revision 16
# speedup vs baseline: 2.0180x; 1.1357x over previous
"""Trainium2 Bass kernel for nn_DeltaFlowLoss (DeFlow-style scene-flow loss).

Strategy (data-parallel over points, 8 cores):
  - Each core streams its slice of points as [128 partitions, T point-columns].
  - Per point: pts_loss, speed, finite-mask, speed-bucket flags, meta one-hots,
    and a masked instance id. Instance ids are split k = 128*h + r; channels
    are duplicated into h0/h1 row blocks so a 128-wide one-hot suffices.
  - Per point-column, a 128-wide instance one-hot (DVE iota-compare bf16;
    some columns built on the Scalar engine as relu(1-|iota-adj|)) is
    contracted with the 14 channel rows on the TensorEngine, accumulating
    [14, 128] instance sums and [14, 6] bucket sums in PSUM.
  - Per-core [14, 134] accumulators go to the host, which does the final
    scalar combination in numpy (exact reference semantics).

Self-contained: hardcodes shapes from the problem spec (N=4M points, K=256
instances, classes < 16, 8 cores).
"""

import sys
import numpy as np

sys.path.insert(0, "/opt/trn_rl_repo")

import ml_dtypes
from contextlib import ExitStack

import concourse.bass as bass
import concourse.bacc as bacc
import concourse.tile as tile
from concourse import mybir

F32 = mybir.dt.float32
BF16 = mybir.dt.bfloat16
I32 = mybir.dt.int32
Alu = mybir.AluOpType
Act = mybir.ActivationFunctionType

N_TOTAL = 4_000_000
N_CORES = 8
K_INST = 256
KH = 128  # one-hot width (instance ids mod 128)
P = 128   # partitions

# Per-core grid: 128 partitions x T point-columns. 8*128*3904 = 3,997,696
# points on-device; the 2,304-point tail is folded in on the host.
T_FULL = 3904
TB_FULL = 488   # point-columns per block (8 blocks)
GR = 8          # one-hot granule (columns per oh tile)
ACT_EVERY = 4   # (unused) legacy
# per-granule one-hot builder: G=gpsimd local_scatter, D=DVE iota-compare,
# A=ScalarE abs+relu
GRANULE_PATTERN = ["G", "D", "G", "D", "G", "A", "G", "G"]

CLASS_WEIGHTS = np.array([0.1, 1.0, 2.0, 2.5, 1.5], dtype=np.float64)

# Base channel slot order (free dim of the BASE tile). The first NCH slots are
# the per-half stationary channels; slots B_M..B_PLHI (consecutive) are the
# bucket-matmul moving columns.
B_SP, B_M0, B_M1, B_M2, B_M3, B_M, B_PL, B_LO, B_PLLO, B_HI, B_PLHI = range(11)
NB = 11
NCH = 7    # channels per half: [sp, moh0..moh3, m, pl]
NS = 14    # stationary rows: channels x {h0, h1}
NY = 6     # bucket cols: [m, pl, lo, pl*lo, hi, pl*hi] = slots B_M..B_PLHI
YS = B_M
# PSUM/host row meaning within a half:
R_SP, R_M0, R_M1, R_M2, R_M3, R_M, R_PL = range(NCH)


def build_program(T=T_FULL, TB=TB_FULL, n_cores=N_CORES):
    assert T % TB == 0 and TB % GR == 0
    nblocks = T // TB
    ngr = TB // GR

    nc = bacc.Bacc("TRN2", target_bir_lowering=False, debug=False,
                   num_devices=n_cores)

    est_d = nc.dram_tensor("est", [P, T * 3], F32, kind="ExternalInput")
    gt_d = nc.dram_tensor("gt", [P, T * 3], F32, kind="ExternalInput")
    cls_d = nc.dram_tensor("cls", [P, T], I32, kind="ExternalInput")
    inst_d = nc.dram_tensor("inst", [P, T], I32, kind="ExternalInput")
    iota_d = nc.dram_tensor("iota", [P, KH], BF16, kind="ExternalInput")
    toff_d = nc.dram_tensor("toff", [P, GR], F32, kind="ExternalInput")
    out_d = nc.dram_tensor("out", [NS, KH], F32, kind="ExternalOutput")
    outb_d = nc.dram_tensor("outb", [NS, NY], F32, kind="ExternalOutput")

    with tile.TileContext(nc) as tc, ExitStack() as ctx:
        const_pool = ctx.enter_context(tc.tile_pool(name="const", bufs=1))
        in_pool = ctx.enter_context(tc.tile_pool(name="inp", bufs=2))
        work_pool = ctx.enter_context(tc.tile_pool(name="work", bufs=2))
        sy_pool = ctx.enter_context(tc.tile_pool(name="sy", bufs=2))
        oh_pool = ctx.enter_context(tc.tile_pool(name="oh", bufs=12))
        psum_pool = ctx.enter_context(
            tc.tile_pool(name="psum", bufs=1, space=bass.MemorySpace.PSUM))
        out_pool = ctx.enter_context(tc.tile_pool(name="outp", bufs=1))

        iota_t = const_pool.tile([P, KH], BF16)
        nc.sync.dma_start(iota_t[:], iota_d[:])
        toff_t = const_pool.tile([P, GR], F32)
        nc.sync.dma_start(toff_t[:], toff_d[:])
        ones_t = const_pool.tile([P, GR], BF16)
        nc.vector.memset(ones_t[:], 1.0)

        biases = {}
        for bv in (640.0, -3.0, -8.5, -12.5, 1.0):
            bt = const_pool.tile([P, 1], F32, tag=f"bias{bv}")
            nc.vector.memset(bt[:], bv)
            biases[bv] = bt

        ps_inst = psum_pool.tile([NS, KH], F32)
        ps_bkt = psum_pool.tile([NS, NY], F32)

        est_v = est_d.ap().rearrange("p (b t c) -> p b t c", b=nblocks, t=TB, c=3)
        gt_v = gt_d.ap().rearrange("p (b t c) -> p b t c", b=nblocks, t=TB, c=3)
        cls_v = cls_d.ap().rearrange("p (b t) -> p b t", b=nblocks, t=TB)
        inst_v = inst_d.ap().rearrange("p (b t) -> p b t", b=nblocks, t=TB)

        for b in range(nblocks):
            est = in_pool.tile([P, TB, 3], F32, tag="est")
            gt = in_pool.tile([P, TB, 3], F32, tag="gt")
            cls_i = in_pool.tile([P, TB], I32, tag="cls")
            inst_i = in_pool.tile([P, TB], I32, tag="inst")
            nc.sync.dma_start(est[:], est_v[:, b])
            nc.sync.dma_start(gt[:], gt_v[:, b])
            nc.sync.dma_start(cls_i[:], cls_v[:, b])
            nc.sync.dma_start(inst_i[:], inst_v[:, b])

            base = work_pool.tile([P, NB, TB], BF16, tag="base")
            sy = sy_pool.tile([P, NS, TB], BF16, tag="sy")

            # --- casts (ACT) ---
            cls_f = work_pool.tile([P, TB], F32, tag="clsf")
            nc.scalar.activation(cls_f[:], cls_i[:], Act.Copy, bias=0.0)
            instf = work_pool.tile([P, TB], F32, tag="instf")  # inst + 640
            nc.scalar.activation(instf[:], inst_i[:], Act.Identity,
                                 bias=biases[640.0][:])

            # --- norms ---
            diff = work_pool.tile([P, TB, 3], F32, tag="diff")
            nc.vector.tensor_tensor(diff[:], est[:], gt[:], Alu.subtract)
            nc.scalar.activation(diff[:], diff[:], Act.Square)
            gt2 = work_pool.tile([P, TB, 3], F32, tag="gt2")
            nc.scalar.activation(gt2[:], gt[:], Act.Square)
            d2s = work_pool.tile([P, TB], F32, tag="d2s")
            nc.vector.tensor_reduce(d2s[:], diff[:], mybir.AxisListType.X, Alu.add)
            gt2s = work_pool.tile([P, TB], F32, tag="gt2s")
            nc.vector.tensor_reduce(gt2s[:], gt2[:], mybir.AxisListType.X, Alu.add)

            # pts_loss / speed (= ||gt||/0.1 = sqrt(100*gt2s))
            nc.scalar.activation(base[:, B_PL], d2s[:], Act.Sqrt)
            nc.scalar.activation(base[:, B_SP], gt2s[:], Act.Sqrt, scale=100.0)

            # --- finite mask ---
            nc.vector.tensor_tensor(d2s[:], d2s[:], gt2s[:], Alu.add)
            nc.vector.tensor_scalar(base[:, B_M], d2s[:], 3.0e38, None, Alu.is_lt)

            # h1 = (inst >= 128); adjm = inst mod 128 for valid points,
            # in [-2048,-1921] for masked ones (negative => ignored by the
            # GPSIMD scatter; never equal to iota 0..127 elsewhere)
            h1 = work_pool.tile([P, TB], BF16, tag="h1")
            nc.vector.tensor_scalar(h1[:], instf[:], 768.0, None, Alu.is_ge)
            adjm = work_pool.tile([P, TB], F32, tag="adjm")
            nc.vector.scalar_tensor_tensor(
                adjm[:], h1[:], -128.0, instf[:], Alu.mult, Alu.add)
            nc.vector.tensor_scalar(adjm[:], adjm[:], -2688.0, None, Alu.add)
            nc.vector.scalar_tensor_tensor(
                adjm[:], base[:, B_M], 2048.0, adjm[:], Alu.mult, Alu.add)

            # --- speed buckets (on squared norm; 0.04^2 and 0.1^2) ---
            nc.vector.tensor_scalar(base[:, B_LO], gt2s[:], 1.6e-3, None, Alu.is_lt)
            nc.vector.tensor_scalar(base[:, B_HI], gt2s[:], 1.0e-2, None, Alu.is_gt)

            # --- meta one-hots (classes 0..15) ---
            # vehicle {7..10,12,13} = (|c-8.5|<=1.5)+(|c-12.5|==0.5)
            # ped {2,3,4} = |c-3|<=1 ; wheeled {6,11} = |c-8.5|==2.5
            a3 = work_pool.tile([P, TB], F32, tag="a3")
            nc.scalar.activation(a3[:], cls_f[:], Act.Abs, bias=biases[-3.0][:])
            a85 = work_pool.tile([P, TB], F32, tag="a85")
            nc.scalar.activation(a85[:], cls_f[:], Act.Abs, bias=biases[-8.5][:])
            a125 = work_pool.tile([P, TB], F32, tag="a125")
            nc.scalar.activation(a125[:], cls_f[:], Act.Abs, bias=biases[-12.5][:])

            nc.vector.tensor_scalar(base[:, B_M0], cls_f[:], 0.0, None, Alu.is_equal)
            nc.vector.tensor_scalar(base[:, B_M2], a3[:], 1.0, None, Alu.is_le)
            nc.vector.tensor_scalar(base[:, B_M3], a85[:], 2.5, None, Alu.is_equal)
            va = work_pool.tile([P, TB], F32, tag="va")
            nc.vector.tensor_scalar(va[:], a85[:], 1.5, None, Alu.is_le)
            nc.vector.scalar_tensor_tensor(
                base[:, B_M1], a125[:], 0.5, va[:], Alu.is_equal, Alu.add)

            nc.vector.tensor_tensor(base[:, B_PLLO], base[:, B_PL],
                                    base[:, B_LO], Alu.mult)
            nc.vector.tensor_tensor(base[:, B_PLHI], base[:, B_PL],
                                    base[:, B_HI], Alu.mult)

            # --- split channels into h0/h1 row blocks ---
            for i in range(NCH):
                nc.vector.tensor_tensor(sy[:, NCH + i], base[:, i], h1[:],
                                        Alu.mult)
                nc.vector.tensor_tensor(sy[:, i], base[:, i], sy[:, NCH + i],
                                        Alu.subtract)

            # --- per-column one-hot + matmuls ---
            for g in range(ngr):
                oh = oh_pool.tile([P, GR, KH], BF16, tag="oh")
                kind = GRANULE_PATTERN[g % len(GRANULE_PATTERN)]
                if kind == "G":
                    idx = oh_pool.tile([P, GR], mybir.dt.int16, tag="gidx")
                    nc.vector.tensor_tensor(
                        idx[:], adjm[:, g * GR:(g + 1) * GR], toff_t[:],
                        Alu.add)
                    nc.gpsimd.local_scatter(
                        oh[:], ones_t[:], idx[:], channels=P,
                        num_elems=GR * KH, num_idxs=GR)
                elif kind == "A":
                    # ScalarE path: |adjm - iota| then relu(1-x) granule-wide
                    for t in range(GR):
                        col = g * GR + t
                        nc.scalar.activation(
                            oh[:, t], iota_t[:], Act.Abs,
                            bias=adjm[:, col:col + 1], scale=-1.0)
                    nc.scalar.activation(
                        oh[:], oh[:], Act.Relu, bias=biases[1.0][:], scale=-1.0)
                else:
                    for t in range(GR):
                        col = g * GR + t
                        nc.vector.tensor_scalar(
                            oh[:, t], iota_t[:], adjm[:, col:col + 1],
                            None, Alu.is_equal)
                for t in range(GR):
                    col = g * GR + t
                    gcol = b * TB + col
                    nc.tensor.matmul(ps_inst[:], sy[:, 0:NS, col], oh[:, t],
                                     start=(gcol == 0), stop=(gcol == T - 1))
                    nc.tensor.matmul(
                        ps_bkt[:], sy[:, 0:NS, col],
                        base[:, YS:YS + NY, col],
                        start=(gcol == 0), stop=(gcol == T - 1))

        out_sb = out_pool.tile([NS, KH], F32)
        nc.vector.tensor_copy(out_sb[:], ps_inst[:])
        nc.sync.dma_start(out_d[:], out_sb[:])
        outb_sb = out_pool.tile([NS, NY], F32)
        nc.vector.tensor_copy(outb_sb[:], ps_bkt[:])
        nc.sync.dma_start(outb_d[:], outb_sb[:])

    nc.compile()
    return nc


# ---------------------------------------------------------------------------
# Host-side helpers
# ---------------------------------------------------------------------------

def np_partials(est, gt, cls, inst, dtype=np.float64):
    """Numpy model of the accumulators for a set of points (row order R_*)."""
    est = est.astype(dtype)
    gt = gt.astype(dtype)
    mask = np.isfinite(est).all(-1) & np.isfinite(gt).all(-1)
    pl = np.where(mask, np.sqrt(((est - gt) ** 2).sum(-1)), 0.0)
    sp = np.where(mask, np.sqrt((gt ** 2).sum(-1)) * 10.0, 0.0)
    g2 = np.where(mask, (gt ** 2).sum(-1), 0.0)
    m = mask.astype(dtype)
    lo = (g2 < 1.6e-3).astype(dtype)
    hi = (g2 > 1.0e-2).astype(dtype)

    e0 = (cls == 0)
    veh = np.isin(cls, [7, 8, 9, 10, 12, 13])
    ped = np.isin(cls, [2, 3, 4])
    whl = np.isin(cls, [6, 11])

    rows = np.stack([sp, e0 * 1.0, veh * 1.0, ped * 1.0, whl * 1.0, m, pl])
    inst_m = np.where(mask, inst, K_INST)
    ioh = np.zeros((len(m), K_INST + 1), dtype)
    ioh[np.arange(len(m)), inst_m] = 1.0
    acc_inst = rows @ ioh[:, 0:K_INST]
    ycols = np.stack([m, pl, lo, pl * lo, hi, pl * hi], axis=1)
    acc_bkt = rows @ ycols
    return {"inst": acc_inst, "bkt": acc_bkt}


def fold_device_out(out, outb):
    """Device out [NS,KH] + outb [4*NS,4*NY] -> {'inst','bkt'} (float64)."""
    out = out.astype(np.float64)
    inst = np.zeros((NCH, K_INST))
    inst[:, 0:KH] = out[0:NCH, 0:KH]
    inst[:, KH:K_INST] = out[NCH:NS, 0:KH]
    bkt14 = outb.astype(np.float64)
    bkt = bkt14[0:NCH] + bkt14[NCH:NS]
    return {"inst": inst, "bkt": bkt}


def combine(acc_inst, acc_bkt):
    """acc_inst [NCH, 256], acc_bkt [NCH, 6] -> scalar loss (float64)."""
    sp_sum = acc_inst[R_SP]
    cnt = acc_inst[R_M]
    pl_sum = acc_inst[R_PL]
    meta_cnt = np.zeros((K_INST, 5))
    for j in range(4):
        meta_cnt[:, j] = acc_inst[R_M0 + j]
    meta_cnt[:, 4] = cnt - meta_cnt[:, 0:4].sum(1)

    def masked_mean(s, c):
        return s / c if c > 0 else 0.0

    def bucket_means(row):
        c_tot, p_tot, c_lo, p_lo, c_hi, p_hi = row
        return (masked_mean(p_lo, c_lo),
                masked_mean(p_tot - p_lo - p_hi, c_tot - c_lo - c_hi),
                masked_mean(p_hi, c_hi))

    mlo, mmid, mhi = bucket_means(acc_bkt[R_M])
    base_loss = mlo + mmid + mhi

    class_loss = 0.0
    meta_rows = [acc_bkt[R_M0 + j] for j in range(4)]
    meta_rows.append(acc_bkt[R_M] - sum(meta_rows))
    for j in range(5):
        l, mm, h = bucket_means(meta_rows[j])
        class_loss += CLASS_WEIGHTS[j] * (0.1 * l + 0.4 * mm + 0.5 * h)

    safe_cnt = np.maximum(cnt, 1.0)
    sp_mean = sp_sum / safe_cnt
    ins_err = np.nan_to_num(pl_sum / safe_cnt, nan=0.0, posinf=0.0, neginf=0.0)
    mode_cls = np.argmax(meta_cnt, axis=1)
    valid = (np.arange(K_INST) > 0) & (cnt > 0) & (sp_mean > 0.4)
    contrib = ins_err * np.exp(ins_err) * CLASS_WEIGHTS[mode_cls]
    n_valid = valid.sum()
    inst_loss = (contrib * valid).sum() / max(n_valid, 1) if n_valid > 0 else 0.0

    return base_loss + class_loss + inst_loss


_NC_CACHE = {}


def _get_program():
    key = (T_FULL, TB_FULL)
    if key not in _NC_CACHE:
        _NC_CACHE[key] = build_program()
    return _NC_CACHE[key]


def make_in_maps(est_flow, gt_flow, gt_classes, gt_instance,
                 T=T_FULL, n_cores=N_CORES):
    npc = P * T
    iota_np = np.broadcast_to(
        np.arange(KH, dtype=ml_dtypes.bfloat16), (P, KH)).copy()
    in_maps = []
    for c in range(n_cores):
        s = slice(c * npc, (c + 1) * npc)
        in_maps.append({
            "est": np.ascontiguousarray(
                est_flow[s].reshape(P, T * 3).astype(np.float32)),
            "gt": np.ascontiguousarray(
                gt_flow[s].reshape(P, T * 3).astype(np.float32)),
            "cls": np.ascontiguousarray(
                gt_classes[s].reshape(P, T).astype(np.int32)),
            "inst": np.ascontiguousarray(
                gt_instance[s].reshape(P, T).astype(np.int32)),
            "iota": iota_np,
            "toff": np.broadcast_to(
                (np.arange(GR) * KH).astype(np.float32), (P, GR)).copy(),
        })
    return in_maps


def kernel(est_flow, gt_flow, gt_classes, gt_instance, _results_hook=None):
    est_flow = np.asarray(est_flow)
    gt_flow = np.asarray(gt_flow)
    gt_classes = np.asarray(gt_classes)
    gt_instance = np.asarray(gt_instance)

    from concourse.bass_utils import run_bass_kernel_spmd

    nc = _get_program()
    in_maps = make_in_maps(est_flow, gt_flow, gt_classes, gt_instance)
    res = run_bass_kernel_spmd(nc, in_maps, core_ids=list(range(N_CORES)))
    if _results_hook is not None:
        _results_hook(res)

    acc_inst = np.zeros((NCH, K_INST))
    acc_bkt = np.zeros((NCH, NY))
    for r in res.results:
        f = fold_device_out(r["out"], r["outb"])
        acc_inst += f["inst"]
        acc_bkt += f["bkt"]

    ndev = N_CORES * P * T_FULL
    if ndev < len(gt_classes):
        s = slice(ndev, None)
        t = np_partials(est_flow[s], gt_flow[s], gt_classes[s], gt_instance[s])
        acc_inst += t["inst"]
        acc_bkt += t["bkt"]

    return np.float32(combine(acc_inst, acc_bkt))


# revision 18
# speedup vs baseline: 2.2882x; 1.1339x over previous
"""Trainium2 Bass kernel for nn_DeltaFlowLoss (DeFlow-style scene-flow loss).

Strategy (data-parallel over points, 8 cores):
  - Each core streams its slice of points as [128 partitions, T point-columns].
  - Per point: pts_loss, speed, finite-mask, speed-bucket flags, meta one-hots,
    and a masked instance id. Instance ids are split k = 128*h + r; channels
    are duplicated into h0/h1 row blocks so a 128-wide one-hot suffices.
  - Per point-column, a 128-wide instance one-hot (DVE iota-compare bf16;
    some columns built on the Scalar engine as relu(1-|iota-adj|)) is
    contracted with the 14 channel rows on the TensorEngine, accumulating
    [14, 128] instance sums and [14, 6] bucket sums in PSUM.
  - Per-core [14, 134] accumulators go to the host, which does the final
    scalar combination in numpy (exact reference semantics).

Self-contained: hardcodes shapes from the problem spec (N=4M points, K=256
instances, classes < 16, 8 cores).
"""

import sys
import numpy as np

sys.path.insert(0, "/opt/trn_rl_repo")

import ml_dtypes
from contextlib import ExitStack

import concourse.bass as bass
import concourse.bacc as bacc
import concourse.tile as tile
from concourse import mybir

F32 = mybir.dt.float32
BF16 = mybir.dt.bfloat16
I32 = mybir.dt.int32
Alu = mybir.AluOpType
Act = mybir.ActivationFunctionType

N_TOTAL = 4_000_000
N_CORES = 8
K_INST = 256
KH = 128  # one-hot width (instance ids mod 128)
P = 128   # partitions

# Per-core grid: 128 partitions x T point-columns. 8*128*3904 = 3,997,696
# points on-device; the 2,304-point tail is folded in on the host.
T_FULL = 3904
TB_FULL = 488   # point-columns per block (8 blocks)
GR = 8          # one-hot granule (columns per oh tile)
ACT_EVERY = 4   # (unused) legacy
# per-granule one-hot builder: G=gpsimd local_scatter, D=DVE iota-compare,
# A=ScalarE abs+relu
GRANULE_PATTERN = ["G", "D", "G", "D", "G", "A", "G", "G"]

CLASS_WEIGHTS = np.array([0.1, 1.0, 2.0, 2.5, 1.5], dtype=np.float64)

# Base channel slot order (free dim of the BASE tile). The first NCH slots are
# the per-half stationary channels; slots B_M..B_PLHI (consecutive) are the
# bucket-matmul moving columns.
B_SP, B_M0, B_M1, B_M2, B_M3, B_M, B_PL, B_LO, B_PLLO, B_HI, B_PLHI = range(11)
NB = 11
NCH = 7    # channels per half: [sp, moh0..moh3, m, pl]
NS = 14    # stationary rows: channels x {h0, h1}
NY = 6     # bucket cols: [m, pl, lo, pl*lo, hi, pl*hi] = slots B_M..B_PLHI
YS = B_M
# PSUM/host row meaning within a half:
R_SP, R_M0, R_M1, R_M2, R_M3, R_M, R_PL = range(NCH)


def build_program(T=T_FULL, TB=TB_FULL, n_cores=N_CORES):
    assert T % TB == 0 and TB % GR == 0
    nblocks = T // TB
    ngr = TB // GR

    nc = bacc.Bacc("TRN2", target_bir_lowering=False, debug=False,
                   num_devices=n_cores)

    est_d = nc.dram_tensor("est", [P, T * 3], F32, kind="ExternalInput")
    gt_d = nc.dram_tensor("gt", [P, T * 3], F32, kind="ExternalInput")
    cls_d = nc.dram_tensor("cls", [P, T], I32, kind="ExternalInput")
    inst_d = nc.dram_tensor("inst", [P, T], I32, kind="ExternalInput")
    iota_d = nc.dram_tensor("iota", [P, KH], BF16, kind="ExternalInput")
    toff_d = nc.dram_tensor("toff", [P, GR], F32, kind="ExternalInput")
    out_d = nc.dram_tensor("out", [NS, KH], F32, kind="ExternalOutput")
    outb_d = nc.dram_tensor("outb", [NS, NY], F32, kind="ExternalOutput")

    with tile.TileContext(nc) as tc, ExitStack() as ctx:
        const_pool = ctx.enter_context(tc.tile_pool(name="const", bufs=1))
        in_pool = ctx.enter_context(tc.tile_pool(name="inp", bufs=2))
        work_pool = ctx.enter_context(tc.tile_pool(name="work", bufs=2))
        sy_pool = ctx.enter_context(tc.tile_pool(name="sy", bufs=2))
        oh_pool = ctx.enter_context(tc.tile_pool(name="oh", bufs=28))
        psum_pool = ctx.enter_context(
            tc.tile_pool(name="psum", bufs=1, space=bass.MemorySpace.PSUM))
        out_pool = ctx.enter_context(tc.tile_pool(name="outp", bufs=1))

        iota_t = const_pool.tile([P, KH], BF16)
        nc.sync.dma_start(iota_t[:], iota_d[:])
        toff_t = const_pool.tile([P, GR], F32)
        nc.sync.dma_start(toff_t[:], toff_d[:])
        ones_t = const_pool.tile([P, GR], BF16)
        nc.vector.memset(ones_t[:], 1.0)

        biases = {}
        for bv in (640.0, -3.0, -8.5, -12.5, 1.0):
            bt = const_pool.tile([P, 1], F32, tag=f"bias{bv}")
            nc.vector.memset(bt[:], bv)
            biases[bv] = bt

        ps_inst = psum_pool.tile([NS, KH], F32)
        ps_bkt = psum_pool.tile([NS, NY], F32)

        est_v = est_d.ap().rearrange("p (b t c) -> p b t c", b=nblocks, t=TB, c=3)
        gt_v = gt_d.ap().rearrange("p (b t c) -> p b t c", b=nblocks, t=TB, c=3)
        cls_v = cls_d.ap().rearrange("p (b t) -> p b t", b=nblocks, t=TB)
        inst_v = inst_d.ap().rearrange("p (b t) -> p b t", b=nblocks, t=TB)

        for b in range(nblocks):
            est = in_pool.tile([P, TB, 3], F32, tag="est")
            gt = in_pool.tile([P, TB, 3], F32, tag="gt")
            cls_i = in_pool.tile([P, TB], I32, tag="cls")
            inst_i = in_pool.tile([P, TB], I32, tag="inst")
            nc.sync.dma_start(est[:], est_v[:, b])
            nc.sync.dma_start(gt[:], gt_v[:, b])
            nc.sync.dma_start(cls_i[:], cls_v[:, b])
            nc.sync.dma_start(inst_i[:], inst_v[:, b])

            base = work_pool.tile([P, NB, TB], BF16, tag="base")
            sy = sy_pool.tile([P, NS, TB], BF16, tag="sy")

            # --- casts (ACT) ---
            cls_f = work_pool.tile([P, TB], F32, tag="clsf")
            nc.scalar.activation(cls_f[:], cls_i[:], Act.Copy, bias=0.0)
            instf = work_pool.tile([P, TB], F32, tag="instf")  # inst + 640
            nc.scalar.activation(instf[:], inst_i[:], Act.Identity,
                                 bias=biases[640.0][:])

            # --- norms ---
            diff = work_pool.tile([P, TB, 3], F32, tag="diff")
            nc.vector.tensor_tensor(diff[:], est[:], gt[:], Alu.subtract)
            nc.scalar.activation(diff[:], diff[:], Act.Square)
            gt2 = work_pool.tile([P, TB, 3], F32, tag="gt2")
            nc.scalar.activation(gt2[:], gt[:], Act.Square)
            d2s = work_pool.tile([P, TB], F32, tag="d2s")
            nc.vector.tensor_reduce(d2s[:], diff[:], mybir.AxisListType.X, Alu.add)
            gt2s = work_pool.tile([P, TB], F32, tag="gt2s")
            nc.vector.tensor_reduce(gt2s[:], gt2[:], mybir.AxisListType.X, Alu.add)

            # pts_loss / speed (= ||gt||/0.1 = sqrt(100*gt2s))
            nc.scalar.activation(base[:, B_PL], d2s[:], Act.Sqrt)
            nc.scalar.activation(base[:, B_SP], gt2s[:], Act.Sqrt, scale=100.0)

            # --- finite mask ---
            nc.vector.tensor_tensor(d2s[:], d2s[:], gt2s[:], Alu.add)
            nc.vector.tensor_scalar(base[:, B_M], d2s[:], 3.0e38, None, Alu.is_lt)

            # h1 = (inst >= 128); adjm = inst mod 128 for valid points,
            # in [-2048,-1921] for masked ones (negative => ignored by the
            # GPSIMD scatter; never equal to iota 0..127 elsewhere)
            h1 = work_pool.tile([P, TB], BF16, tag="h1")
            nc.vector.tensor_scalar(h1[:], instf[:], 768.0, None, Alu.is_ge)
            adjm = work_pool.tile([P, TB], F32, tag="adjm")
            nc.vector.scalar_tensor_tensor(
                adjm[:], h1[:], -128.0, instf[:], Alu.mult, Alu.add)
            nc.vector.tensor_scalar(adjm[:], adjm[:], -2688.0, None, Alu.add)
            nc.vector.scalar_tensor_tensor(
                adjm[:], base[:, B_M], 2048.0, adjm[:], Alu.mult, Alu.add)

            # --- speed buckets (on squared norm; 0.04^2 and 0.1^2) ---
            nc.vector.tensor_scalar(base[:, B_LO], gt2s[:], 1.6e-3, None, Alu.is_lt)
            nc.vector.tensor_scalar(base[:, B_HI], gt2s[:], 1.0e-2, None, Alu.is_gt)

            # --- meta one-hots (classes 0..15) ---
            # vehicle {7..10,12,13} = (|c-8.5|<=1.5)+(|c-12.5|==0.5)
            # ped {2,3,4} = |c-3|<=1 ; wheeled {6,11} = |c-8.5|==2.5
            a3 = work_pool.tile([P, TB], F32, tag="a3")
            nc.scalar.activation(a3[:], cls_f[:], Act.Abs, bias=biases[-3.0][:])
            a85 = work_pool.tile([P, TB], F32, tag="a85")
            nc.scalar.activation(a85[:], cls_f[:], Act.Abs, bias=biases[-8.5][:])
            a125 = work_pool.tile([P, TB], F32, tag="a125")
            nc.scalar.activation(a125[:], cls_f[:], Act.Abs, bias=biases[-12.5][:])

            nc.vector.tensor_scalar(base[:, B_M0], cls_f[:], 0.0, None, Alu.is_equal)
            nc.vector.tensor_scalar(base[:, B_M2], a3[:], 1.0, None, Alu.is_le)
            nc.vector.tensor_scalar(base[:, B_M3], a85[:], 2.5, None, Alu.is_equal)
            va = work_pool.tile([P, TB], F32, tag="va")
            nc.vector.tensor_scalar(va[:], a85[:], 1.5, None, Alu.is_le)
            nc.vector.scalar_tensor_tensor(
                base[:, B_M1], a125[:], 0.5, va[:], Alu.is_equal, Alu.add)

            nc.vector.tensor_tensor(base[:, B_PLLO], base[:, B_PL],
                                    base[:, B_LO], Alu.mult)
            nc.vector.tensor_tensor(base[:, B_PLHI], base[:, B_PL],
                                    base[:, B_HI], Alu.mult)

            # --- split channels into h0/h1 row blocks ---
            for i in range(NCH):
                nc.vector.tensor_tensor(sy[:, NCH + i], base[:, i], h1[:],
                                        Alu.mult)
                nc.vector.tensor_tensor(sy[:, i], base[:, i], sy[:, NCH + i],
                                        Alu.subtract)

            # --- per-column one-hot + matmuls ---
            for g in range(ngr):
                oh = oh_pool.tile([P, GR, KH], BF16, tag="oh")
                kind = GRANULE_PATTERN[g % len(GRANULE_PATTERN)]
                if kind == "G":
                    idx = oh_pool.tile([P, GR], mybir.dt.int16, tag="gidx")
                    nc.vector.tensor_tensor(
                        idx[:], adjm[:, g * GR:(g + 1) * GR], toff_t[:],
                        Alu.add)
                    nc.gpsimd.local_scatter(
                        oh[:], ones_t[:], idx[:], channels=P,
                        num_elems=GR * KH, num_idxs=GR)
                elif kind == "A":
                    # ScalarE path: |adjm - iota| then relu(1-x) granule-wide
                    for t in range(GR):
                        col = g * GR + t
                        nc.scalar.activation(
                            oh[:, t], iota_t[:], Act.Abs,
                            bias=adjm[:, col:col + 1], scale=-1.0)
                    nc.scalar.activation(
                        oh[:], oh[:], Act.Relu, bias=biases[1.0][:], scale=-1.0)
                else:
                    for t in range(GR):
                        col = g * GR + t
                        nc.vector.tensor_scalar(
                            oh[:, t], iota_t[:], adjm[:, col:col + 1],
                            None, Alu.is_equal)
                for t in range(GR):
                    col = g * GR + t
                    gcol = b * TB + col
                    nc.tensor.matmul(ps_inst[:], sy[:, 0:NS, col], oh[:, t],
                                     start=(gcol == 0), stop=(gcol == T - 1))
                    nc.tensor.matmul(
                        ps_bkt[:], sy[:, 0:NS, col],
                        base[:, YS:YS + NY, col],
                        start=(gcol == 0), stop=(gcol == T - 1))

        out_sb = out_pool.tile([NS, KH], F32)
        nc.vector.tensor_copy(out_sb[:], ps_inst[:])
        nc.sync.dma_start(out_d[:], out_sb[:])
        outb_sb = out_pool.tile([NS, NY], F32)
        nc.vector.tensor_copy(outb_sb[:], ps_bkt[:])
        nc.sync.dma_start(outb_d[:], outb_sb[:])

    nc.compile()
    return nc


# ---------------------------------------------------------------------------
# Host-side helpers
# ---------------------------------------------------------------------------

def np_partials(est, gt, cls, inst, dtype=np.float64):
    """Numpy model of the accumulators for a set of points (row order R_*)."""
    est = est.astype(dtype)
    gt = gt.astype(dtype)
    mask = np.isfinite(est).all(-1) & np.isfinite(gt).all(-1)
    pl = np.where(mask, np.sqrt(((est - gt) ** 2).sum(-1)), 0.0)
    sp = np.where(mask, np.sqrt((gt ** 2).sum(-1)) * 10.0, 0.0)
    g2 = np.where(mask, (gt ** 2).sum(-1), 0.0)
    m = mask.astype(dtype)
    lo = (g2 < 1.6e-3).astype(dtype)
    hi = (g2 > 1.0e-2).astype(dtype)

    e0 = (cls == 0)
    veh = np.isin(cls, [7, 8, 9, 10, 12, 13])
    ped = np.isin(cls, [2, 3, 4])
    whl = np.isin(cls, [6, 11])

    rows = np.stack([sp, e0 * 1.0, veh * 1.0, ped * 1.0, whl * 1.0, m, pl])
    inst_m = np.where(mask, inst, K_INST)
    ioh = np.zeros((len(m), K_INST + 1), dtype)
    ioh[np.arange(len(m)), inst_m] = 1.0
    acc_inst = rows @ ioh[:, 0:K_INST]
    ycols = np.stack([m, pl, lo, pl * lo, hi, pl * hi], axis=1)
    acc_bkt = rows @ ycols
    return {"inst": acc_inst, "bkt": acc_bkt}


def fold_device_out(out, outb):
    """Device out [NS,KH] + outb [4*NS,4*NY] -> {'inst','bkt'} (float64)."""
    out = out.astype(np.float64)
    inst = np.zeros((NCH, K_INST))
    inst[:, 0:KH] = out[0:NCH, 0:KH]
    inst[:, KH:K_INST] = out[NCH:NS, 0:KH]
    bkt14 = outb.astype(np.float64)
    bkt = bkt14[0:NCH] + bkt14[NCH:NS]
    return {"inst": inst, "bkt": bkt}


def combine(acc_inst, acc_bkt):
    """acc_inst [NCH, 256], acc_bkt [NCH, 6] -> scalar loss (float64)."""
    sp_sum = acc_inst[R_SP]
    cnt = acc_inst[R_M]
    pl_sum = acc_inst[R_PL]
    meta_cnt = np.zeros((K_INST, 5))
    for j in range(4):
        meta_cnt[:, j] = acc_inst[R_M0 + j]
    meta_cnt[:, 4] = cnt - meta_cnt[:, 0:4].sum(1)

    def masked_mean(s, c):
        return s / c if c > 0 else 0.0

    def bucket_means(row):
        c_tot, p_tot, c_lo, p_lo, c_hi, p_hi = row
        return (masked_mean(p_lo, c_lo),
                masked_mean(p_tot - p_lo - p_hi, c_tot - c_lo - c_hi),
                masked_mean(p_hi, c_hi))

    mlo, mmid, mhi = bucket_means(acc_bkt[R_M])
    base_loss = mlo + mmid + mhi

    class_loss = 0.0
    meta_rows = [acc_bkt[R_M0 + j] for j in range(4)]
    meta_rows.append(acc_bkt[R_M] - sum(meta_rows))
    for j in range(5):
        l, mm, h = bucket_means(meta_rows[j])
        class_loss += CLASS_WEIGHTS[j] * (0.1 * l + 0.4 * mm + 0.5 * h)

    safe_cnt = np.maximum(cnt, 1.0)
    sp_mean = sp_sum / safe_cnt
    ins_err = np.nan_to_num(pl_sum / safe_cnt, nan=0.0, posinf=0.0, neginf=0.0)
    mode_cls = np.argmax(meta_cnt, axis=1)
    valid = (np.arange(K_INST) > 0) & (cnt > 0) & (sp_mean > 0.4)
    contrib = ins_err * np.exp(ins_err) * CLASS_WEIGHTS[mode_cls]
    n_valid = valid.sum()
    inst_loss = (contrib * valid).sum() / max(n_valid, 1) if n_valid > 0 else 0.0

    return base_loss + class_loss + inst_loss


_NC_CACHE = {}


def _get_program():
    key = (T_FULL, TB_FULL)
    if key not in _NC_CACHE:
        _NC_CACHE[key] = build_program()
    return _NC_CACHE[key]


def make_in_maps(est_flow, gt_flow, gt_classes, gt_instance,
                 T=T_FULL, n_cores=N_CORES):
    npc = P * T
    iota_np = np.broadcast_to(
        np.arange(KH, dtype=ml_dtypes.bfloat16), (P, KH)).copy()
    in_maps = []
    for c in range(n_cores):
        s = slice(c * npc, (c + 1) * npc)
        in_maps.append({
            "est": np.ascontiguousarray(
                est_flow[s].reshape(P, T * 3).astype(np.float32)),
            "gt": np.ascontiguousarray(
                gt_flow[s].reshape(P, T * 3).astype(np.float32)),
            "cls": np.ascontiguousarray(
                gt_classes[s].reshape(P, T).astype(np.int32)),
            "inst": np.ascontiguousarray(
                gt_instance[s].reshape(P, T).astype(np.int32)),
            "iota": iota_np,
            "toff": np.broadcast_to(
                (np.arange(GR) * KH).astype(np.float32), (P, GR)).copy(),
        })
    return in_maps


def kernel(est_flow, gt_flow, gt_classes, gt_instance, _results_hook=None):
    est_flow = np.asarray(est_flow)
    gt_flow = np.asarray(gt_flow)
    gt_classes = np.asarray(gt_classes)
    gt_instance = np.asarray(gt_instance)

    from concourse.bass_utils import run_bass_kernel_spmd

    nc = _get_program()
    in_maps = make_in_maps(est_flow, gt_flow, gt_classes, gt_instance)
    res = run_bass_kernel_spmd(nc, in_maps, core_ids=list(range(N_CORES)))
    if _results_hook is not None:
        _results_hook(res)

    acc_inst = np.zeros((NCH, K_INST))
    acc_bkt = np.zeros((NCH, NY))
    for r in res.results:
        f = fold_device_out(r["out"], r["outb"])
        acc_inst += f["inst"]
        acc_bkt += f["bkt"]

    ndev = N_CORES * P * T_FULL
    if ndev < len(gt_classes):
        s = slice(ndev, None)
        t = np_partials(est_flow[s], gt_flow[s], gt_classes[s], gt_instance[s])
        acc_inst += t["inst"]
        acc_bkt += t["bkt"]

    return np.float32(combine(acc_inst, acc_bkt))


# revision 22
# speedup vs baseline: 2.2998x; 1.0051x over previous
"""Trainium2 Bass kernel for nn_DeltaFlowLoss (DeFlow-style scene-flow loss).

Strategy (data-parallel over points, 8 cores):
  - Each core streams its slice of points as [128 partitions, T point-columns].
  - Per point: pts_loss, speed, finite-mask, speed-bucket flags, meta one-hots,
    and a masked instance id. Instance ids are split k = 128*h + r; channels
    are duplicated into h0/h1 row blocks so a 128-wide one-hot suffices.
  - Per point-column, a 128-wide instance one-hot (DVE iota-compare bf16;
    some columns built on the Scalar engine as relu(1-|iota-adj|)) is
    contracted with the 14 channel rows on the TensorEngine, accumulating
    [14, 128] instance sums and [14, 6] bucket sums in PSUM.
  - Per-core [14, 134] accumulators go to the host, which does the final
    scalar combination in numpy (exact reference semantics).

Self-contained: hardcodes shapes from the problem spec (N=4M points, K=256
instances, classes < 16, 8 cores).
"""

import sys
import numpy as np

sys.path.insert(0, "/opt/trn_rl_repo")

import ml_dtypes
from contextlib import ExitStack

import concourse.bass as bass
import concourse.bacc as bacc
import concourse.tile as tile
from concourse import mybir

F32 = mybir.dt.float32
BF16 = mybir.dt.bfloat16
I32 = mybir.dt.int32
Alu = mybir.AluOpType
Act = mybir.ActivationFunctionType

N_TOTAL = 4_000_000
N_CORES = 8
K_INST = 256
KH = 128  # one-hot width (instance ids mod 128)
P = 128   # partitions

# Per-core grid: 128 partitions x T point-columns. 8*128*3904 = 3,997,696
# points on-device; the 2,304-point tail is folded in on the host.
T_FULL = 3904
TB_FULL = 488   # point-columns per block (8 blocks)
GR = 8          # one-hot granule (columns per oh tile)
ACT_EVERY = 4   # (unused) legacy
# per-granule one-hot builder: G=gpsimd local_scatter, D=DVE iota-compare,
# A=ScalarE abs+relu
GRANULE_PATTERN = ["G", "D", "G", "D", "G", "A", "G", "G"]

CLASS_WEIGHTS = np.array([0.1, 1.0, 2.0, 2.5, 1.5], dtype=np.float64)

# Base channel slot order (free dim of the BASE tile). The first NCH slots are
# the per-half stationary channels; slots B_M..B_PLHI (consecutive) are the
# bucket-matmul moving columns.
B_SP, B_M0, B_M1, B_M2, B_M3, B_M, B_PL, B_LO, B_PLLO, B_HI, B_PLHI = range(11)
NB = 11
NCH = 7    # channels per half: [sp, moh0..moh3, m, pl]
NS = 14    # stationary rows: channels x {h0, h1}
NY = 6     # bucket cols: [m, pl, lo, pl*lo, hi, pl*hi] = slots B_M..B_PLHI
YS = B_M
# PSUM/host row meaning within a half:
R_SP, R_M0, R_M1, R_M2, R_M3, R_M, R_PL = range(NCH)


def build_program(T=T_FULL, TB=TB_FULL, n_cores=N_CORES):
    assert T % TB == 0 and TB % GR == 0
    nblocks = T // TB
    ngr = TB // GR

    nc = bacc.Bacc("TRN2", target_bir_lowering=False, debug=False,
                   num_devices=n_cores)

    est_d = nc.dram_tensor("est", [P, T * 3], F32, kind="ExternalInput")
    gt_d = nc.dram_tensor("gt", [P, T * 3], F32, kind="ExternalInput")
    cls_d = nc.dram_tensor("cls", [P, T], I32, kind="ExternalInput")
    inst_d = nc.dram_tensor("inst", [P, T], I32, kind="ExternalInput")
    iota_d = nc.dram_tensor("iota", [P, KH], BF16, kind="ExternalInput")
    toff_d = nc.dram_tensor("toff", [P, GR], F32, kind="ExternalInput")
    out_d = nc.dram_tensor("out", [NS, KH], F32, kind="ExternalOutput")
    outb_d = nc.dram_tensor("outb", [NS, NY], F32, kind="ExternalOutput")

    with tile.TileContext(nc) as tc, ExitStack() as ctx:
        const_pool = ctx.enter_context(tc.tile_pool(name="const", bufs=1))
        in_pool = ctx.enter_context(tc.tile_pool(name="inp", bufs=2))
        work_pool = ctx.enter_context(tc.tile_pool(name="work", bufs=2))
        sy_pool = ctx.enter_context(tc.tile_pool(name="sy", bufs=3))
        oh_pool = ctx.enter_context(tc.tile_pool(name="oh", bufs=28))
        psum_pool = ctx.enter_context(
            tc.tile_pool(name="psum", bufs=1, space=bass.MemorySpace.PSUM))
        out_pool = ctx.enter_context(tc.tile_pool(name="outp", bufs=1))

        iota_t = const_pool.tile([P, KH], BF16)
        nc.sync.dma_start(iota_t[:], iota_d[:])
        toff_t = const_pool.tile([P, GR], F32)
        nc.sync.dma_start(toff_t[:], toff_d[:])
        ones_t = const_pool.tile([P, GR], BF16)
        nc.vector.memset(ones_t[:], 1.0)

        biases = {}
        for bv in (640.0, -3.0, -8.5, -12.5, 1.0):
            bt = const_pool.tile([P, 1], F32, tag=f"bias{bv}")
            nc.vector.memset(bt[:], bv)
            biases[bv] = bt

        ps_inst = psum_pool.tile([NS, KH], F32)
        ps_bkt = psum_pool.tile([NS, NY], F32)

        est_v = est_d.ap().rearrange("p (b t c) -> p b t c", b=nblocks, t=TB, c=3)
        gt_v = gt_d.ap().rearrange("p (b t c) -> p b t c", b=nblocks, t=TB, c=3)
        cls_v = cls_d.ap().rearrange("p (b t) -> p b t", b=nblocks, t=TB)
        inst_v = inst_d.ap().rearrange("p (b t) -> p b t", b=nblocks, t=TB)

        for b in range(nblocks):
            est = in_pool.tile([P, TB, 3], F32, tag="est")
            gt = in_pool.tile([P, TB, 3], F32, tag="gt")
            cls_i = in_pool.tile([P, TB], I32, tag="cls")
            inst_i = in_pool.tile([P, TB], I32, tag="inst")
            nc.sync.dma_start(est[:], est_v[:, b])
            nc.sync.dma_start(gt[:], gt_v[:, b])
            nc.sync.dma_start(cls_i[:], cls_v[:, b])
            nc.sync.dma_start(inst_i[:], inst_v[:, b])

            base = work_pool.tile([P, NB, TB], BF16, tag="base")
            sy = sy_pool.tile([P, NS, TB], BF16, tag="sy")

            # --- casts (ACT) ---
            cls_f = work_pool.tile([P, TB], F32, tag="clsf")
            nc.scalar.activation(cls_f[:], cls_i[:], Act.Copy, bias=0.0)
            instf = work_pool.tile([P, TB], F32, tag="instf")  # inst + 640
            nc.scalar.activation(instf[:], inst_i[:], Act.Identity,
                                 bias=biases[640.0][:])

            # --- norms (in-place over the spent input tiles) ---
            nc.vector.tensor_tensor(est[:], est[:], gt[:], Alu.subtract)
            nc.scalar.activation(est[:], est[:], Act.Square)
            nc.scalar.activation(gt[:], gt[:], Act.Square)
            d2s = work_pool.tile([P, TB], F32, tag="d2s")
            nc.vector.tensor_reduce(d2s[:], est[:], mybir.AxisListType.X, Alu.add)
            gt2s = work_pool.tile([P, TB], F32, tag="gt2s")
            nc.vector.tensor_reduce(gt2s[:], gt[:], mybir.AxisListType.X, Alu.add)

            # pts_loss / speed (= ||gt||/0.1 = sqrt(100*gt2s))
            nc.scalar.activation(base[:, B_PL], d2s[:], Act.Sqrt)
            nc.scalar.activation(base[:, B_SP], gt2s[:], Act.Sqrt, scale=100.0)

            # --- finite mask ---
            nc.vector.tensor_tensor(d2s[:], d2s[:], gt2s[:], Alu.add)
            nc.vector.tensor_scalar(base[:, B_M], d2s[:], 3.0e38, None, Alu.is_lt)

            # h1 = (inst >= 128); adjm = inst mod 128 for valid points,
            # in [-2048,-1921] for masked ones (negative => ignored by the
            # GPSIMD scatter; never equal to iota 0..127 elsewhere)
            h1 = work_pool.tile([P, TB], BF16, tag="h1")
            nc.vector.tensor_scalar(h1[:], instf[:], 768.0, None, Alu.is_ge)
            adjm = work_pool.tile([P, TB], F32, tag="adjm")
            nc.vector.scalar_tensor_tensor(
                adjm[:], h1[:], -128.0, instf[:], Alu.mult, Alu.add)
            nc.vector.tensor_scalar(adjm[:], adjm[:], -2688.0, None, Alu.add)
            nc.vector.scalar_tensor_tensor(
                adjm[:], base[:, B_M], 2048.0, adjm[:], Alu.mult, Alu.add)

            # --- speed buckets (on squared norm; 0.04^2 and 0.1^2) ---
            nc.vector.tensor_scalar(base[:, B_LO], gt2s[:], 1.6e-3, None, Alu.is_lt)
            nc.vector.tensor_scalar(base[:, B_HI], gt2s[:], 1.0e-2, None, Alu.is_gt)

            # --- meta one-hots (classes 0..15) ---
            # vehicle {7..10,12,13} = (|c-8.5|<=1.5)+(|c-12.5|==0.5)
            # ped {2,3,4} = |c-3|<=1 ; wheeled {6,11} = |c-8.5|==2.5
            a3 = work_pool.tile([P, TB], F32, tag="a3")
            nc.scalar.activation(a3[:], cls_f[:], Act.Abs, bias=biases[-3.0][:])
            a85 = work_pool.tile([P, TB], F32, tag="a85")
            nc.scalar.activation(a85[:], cls_f[:], Act.Abs, bias=biases[-8.5][:])
            a125 = work_pool.tile([P, TB], F32, tag="a125")
            nc.scalar.activation(a125[:], cls_f[:], Act.Abs, bias=biases[-12.5][:])

            nc.vector.tensor_scalar(base[:, B_M0], cls_f[:], 0.0, None, Alu.is_equal)
            nc.vector.tensor_scalar(base[:, B_M2], a3[:], 1.0, None, Alu.is_le)
            nc.vector.tensor_scalar(base[:, B_M3], a85[:], 2.5, None, Alu.is_equal)
            va = work_pool.tile([P, TB], F32, tag="va")
            nc.vector.tensor_scalar(va[:], a85[:], 1.5, None, Alu.is_le)
            nc.vector.scalar_tensor_tensor(
                base[:, B_M1], a125[:], 0.5, va[:], Alu.is_equal, Alu.add)

            nc.vector.tensor_tensor(base[:, B_PLLO], base[:, B_PL],
                                    base[:, B_LO], Alu.mult)
            nc.vector.tensor_tensor(base[:, B_PLHI], base[:, B_PL],
                                    base[:, B_HI], Alu.mult)

            # --- split channels into h0/h1 row blocks ---
            for i in range(NCH):
                nc.vector.tensor_tensor(sy[:, NCH + i], base[:, i], h1[:],
                                        Alu.mult)
                nc.vector.tensor_tensor(sy[:, i], base[:, i], sy[:, NCH + i],
                                        Alu.subtract)

            # --- per-column one-hot + matmuls ---
            for g in range(ngr):
                oh = oh_pool.tile([P, GR, KH], BF16, tag="oh")
                kind = GRANULE_PATTERN[g % len(GRANULE_PATTERN)]
                if kind == "G":
                    idx = oh_pool.tile([P, GR], mybir.dt.int16, tag="gidx")
                    nc.vector.tensor_tensor(
                        idx[:], adjm[:, g * GR:(g + 1) * GR], toff_t[:],
                        Alu.add)
                    nc.gpsimd.local_scatter(
                        oh[:], ones_t[:], idx[:], channels=P,
                        num_elems=GR * KH, num_idxs=GR)
                elif kind == "A":
                    # ScalarE path: |adjm - iota| then relu(1-x) granule-wide
                    for t in range(GR):
                        col = g * GR + t
                        nc.scalar.activation(
                            oh[:, t], iota_t[:], Act.Abs,
                            bias=adjm[:, col:col + 1], scale=-1.0)
                    nc.scalar.activation(
                        oh[:], oh[:], Act.Relu, bias=biases[1.0][:], scale=-1.0)
                else:
                    for t in range(GR):
                        col = g * GR + t
                        nc.vector.tensor_scalar(
                            oh[:, t], iota_t[:], adjm[:, col:col + 1],
                            None, Alu.is_equal)
                for t in range(GR):
                    col = g * GR + t
                    gcol = b * TB + col
                    nc.tensor.matmul(ps_inst[:], sy[:, 0:NS, col], oh[:, t],
                                     start=(gcol == 0), stop=(gcol == T - 1))
                    nc.tensor.matmul(
                        ps_bkt[:], sy[:, 0:NS, col],
                        base[:, YS:YS + NY, col],
                        start=(gcol == 0), stop=(gcol == T - 1))

        out_sb = out_pool.tile([NS, KH], F32)
        nc.vector.tensor_copy(out_sb[:], ps_inst[:])
        nc.sync.dma_start(out_d[:], out_sb[:])
        outb_sb = out_pool.tile([NS, NY], F32)
        nc.vector.tensor_copy(outb_sb[:], ps_bkt[:])
        nc.sync.dma_start(outb_d[:], outb_sb[:])

    nc.compile()
    return nc


# ---------------------------------------------------------------------------
# Host-side helpers
# ---------------------------------------------------------------------------

def np_partials(est, gt, cls, inst, dtype=np.float64):
    """Numpy model of the accumulators for a set of points (row order R_*)."""
    est = est.astype(dtype)
    gt = gt.astype(dtype)
    mask = np.isfinite(est).all(-1) & np.isfinite(gt).all(-1)
    pl = np.where(mask, np.sqrt(((est - gt) ** 2).sum(-1)), 0.0)
    sp = np.where(mask, np.sqrt((gt ** 2).sum(-1)) * 10.0, 0.0)
    g2 = np.where(mask, (gt ** 2).sum(-1), 0.0)
    m = mask.astype(dtype)
    lo = (g2 < 1.6e-3).astype(dtype)
    hi = (g2 > 1.0e-2).astype(dtype)

    e0 = (cls == 0)
    veh = np.isin(cls, [7, 8, 9, 10, 12, 13])
    ped = np.isin(cls, [2, 3, 4])
    whl = np.isin(cls, [6, 11])

    rows = np.stack([sp, e0 * 1.0, veh * 1.0, ped * 1.0, whl * 1.0, m, pl])
    inst_m = np.where(mask, inst, K_INST)
    ioh = np.zeros((len(m), K_INST + 1), dtype)
    ioh[np.arange(len(m)), inst_m] = 1.0
    acc_inst = rows @ ioh[:, 0:K_INST]
    ycols = np.stack([m, pl, lo, pl * lo, hi, pl * hi], axis=1)
    acc_bkt = rows @ ycols
    return {"inst": acc_inst, "bkt": acc_bkt}


def fold_device_out(out, outb):
    """Device out [NS,KH] + outb [4*NS,4*NY] -> {'inst','bkt'} (float64)."""
    out = out.astype(np.float64)
    inst = np.zeros((NCH, K_INST))
    inst[:, 0:KH] = out[0:NCH, 0:KH]
    inst[:, KH:K_INST] = out[NCH:NS, 0:KH]
    bkt14 = outb.astype(np.float64)
    bkt = bkt14[0:NCH] + bkt14[NCH:NS]
    return {"inst": inst, "bkt": bkt}


def combine(acc_inst, acc_bkt):
    """acc_inst [NCH, 256], acc_bkt [NCH, 6] -> scalar loss (float64)."""
    sp_sum = acc_inst[R_SP]
    cnt = acc_inst[R_M]
    pl_sum = acc_inst[R_PL]
    meta_cnt = np.zeros((K_INST, 5))
    for j in range(4):
        meta_cnt[:, j] = acc_inst[R_M0 + j]
    meta_cnt[:, 4] = cnt - meta_cnt[:, 0:4].sum(1)

    def masked_mean(s, c):
        return s / c if c > 0 else 0.0

    def bucket_means(row):
        c_tot, p_tot, c_lo, p_lo, c_hi, p_hi = row
        return (masked_mean(p_lo, c_lo),
                masked_mean(p_tot - p_lo - p_hi, c_tot - c_lo - c_hi),
                masked_mean(p_hi, c_hi))

    mlo, mmid, mhi = bucket_means(acc_bkt[R_M])
    base_loss = mlo + mmid + mhi

    class_loss = 0.0
    meta_rows = [acc_bkt[R_M0 + j] for j in range(4)]
    meta_rows.append(acc_bkt[R_M] - sum(meta_rows))
    for j in range(5):
        l, mm, h = bucket_means(meta_rows[j])
        class_loss += CLASS_WEIGHTS[j] * (0.1 * l + 0.4 * mm + 0.5 * h)

    safe_cnt = np.maximum(cnt, 1.0)
    sp_mean = sp_sum / safe_cnt
    ins_err = np.nan_to_num(pl_sum / safe_cnt, nan=0.0, posinf=0.0, neginf=0.0)
    mode_cls = np.argmax(meta_cnt, axis=1)
    valid = (np.arange(K_INST) > 0) & (cnt > 0) & (sp_mean > 0.4)
    contrib = ins_err * np.exp(ins_err) * CLASS_WEIGHTS[mode_cls]
    n_valid = valid.sum()
    inst_loss = (contrib * valid).sum() / max(n_valid, 1) if n_valid > 0 else 0.0

    return base_loss + class_loss + inst_loss


_NC_CACHE = {}


def _get_program():
    key = (T_FULL, TB_FULL)
    if key not in _NC_CACHE:
        _NC_CACHE[key] = build_program()
    return _NC_CACHE[key]


def make_in_maps(est_flow, gt_flow, gt_classes, gt_instance,
                 T=T_FULL, n_cores=N_CORES):
    npc = P * T
    iota_np = np.broadcast_to(
        np.arange(KH, dtype=ml_dtypes.bfloat16), (P, KH)).copy()
    in_maps = []
    for c in range(n_cores):
        s = slice(c * npc, (c + 1) * npc)
        in_maps.append({
            "est": np.ascontiguousarray(
                est_flow[s].reshape(P, T * 3).astype(np.float32)),
            "gt": np.ascontiguousarray(
                gt_flow[s].reshape(P, T * 3).astype(np.float32)),
            "cls": np.ascontiguousarray(
                gt_classes[s].reshape(P, T).astype(np.int32)),
            "inst": np.ascontiguousarray(
                gt_instance[s].reshape(P, T).astype(np.int32)),
            "iota": iota_np,
            "toff": np.broadcast_to(
                (np.arange(GR) * KH).astype(np.float32), (P, GR)).copy(),
        })
    return in_maps


def kernel(est_flow, gt_flow, gt_classes, gt_instance, _results_hook=None):
    est_flow = np.asarray(est_flow)
    gt_flow = np.asarray(gt_flow)
    gt_classes = np.asarray(gt_classes)
    gt_instance = np.asarray(gt_instance)

    from concourse.bass_utils import run_bass_kernel_spmd

    nc = _get_program()
    in_maps = make_in_maps(est_flow, gt_flow, gt_classes, gt_instance)
    res = run_bass_kernel_spmd(nc, in_maps, core_ids=list(range(N_CORES)))
    if _results_hook is not None:
        _results_hook(res)

    acc_inst = np.zeros((NCH, K_INST))
    acc_bkt = np.zeros((NCH, NY))
    for r in res.results:
        f = fold_device_out(r["out"], r["outb"])
        acc_inst += f["inst"]
        acc_bkt += f["bkt"]

    ndev = N_CORES * P * T_FULL
    if ndev < len(gt_classes):
        s = slice(ndev, None)
        t = np_partials(est_flow[s], gt_flow[s], gt_classes[s], gt_instance[s])
        acc_inst += t["inst"]
        acc_bkt += t["bkt"]

    return np.float32(combine(acc_inst, acc_bkt))
